# revision 5
# baseline (speedup 1.0000x reference)
"""Trainium2 Bass kernel for the Tolles-Lawson custom loss.

reference:
    c = model_output[:, :18]; d = model_output[:, 18:19]
    tmp = sum(A * (beta_TL + c), axis=1, keepdims=True) + d
    L = mean((tmp - y)^2) + mean((tmp - B_tl)^2)

Strategy: pure data parallel over rows on 8 cores. Each core gets
R = 501,760 rows (core 7 zero-padded; zero rows contribute 0 to both
sums). Rows are block-assigned to partitions so every DMA reads a
contiguous per-partition run (~20KB). Inputs are cast f32->bf16 during
the SWDGE DMA (halves SBUF, enables 2x DVE modes); all reductions
accumulate in f32. Per 128xT tile:
    bc   = mo + [beta,0]            (DVE, bf16 2x)
    a19  = [A | 1.0] via ACT copy into a ones-preset tile
    bc  *= a19                      (DVE, bf16 2x, in-place)
    tmp  = reduce_sum(bc, per-row)  (DVE, f32 out)  == A.(beta+c) + d
    e    = tmp - y ; acc[2i]   = sum(e*e)   (tensor_tensor_reduce)
    e2   = tmp - B ; acc[2i+1] = sum(e2*e2)
Each core returns acc [128, 2*NT] f32 partial sums; the host sums and
divides by N (the cross-shard all-reduce of the MSE sums).
"""

import numpy as np
import ml_dtypes

import concourse.bacc as bacc
import concourse.mybir as mybir
from concourse import tile
from concourse.bass_utils import run_bass_kernel_spmd

N_TOTAL = 4_000_000
NCOEF = 18
C = NCOEF + 1  # 19: coeffs + bias column
P = 128
T = 280          # rows per partition per tile
NT = 14          # tiles per core
RP = T * NT      # 3920 rows per partition
R = P * RP       # 501,760 rows per core
N_CORES = 8

f32 = mybir.dt.float32
bf16 = mybir.dt.bfloat16

USE_BF16 = True

_cached = {}


def _build_nc(T=T, NT=NT, use_bf16=None):
    key = ("nc", T, NT, use_bf16)
    if key in _cached:
        return _cached[key]
    dt_in = bf16 if (USE_BF16 if use_bf16 is None else use_bf16) else f32
    nc = bacc.Bacc(None)
    mo_ext = nc.declare_dram_parameter("mo", [NT, P, T * C], f32, isOutput=False)
    a_ext = nc.declare_dram_parameter("a", [NT, P, T * NCOEF], f32, isOutput=False)
    y_ext = nc.declare_dram_parameter("y", [NT, P, T], f32, isOutput=False)
    b_ext = nc.declare_dram_parameter("b", [NT, P, T], f32, isOutput=False)
    vb_ext = nc.declare_dram_parameter("vb", [P, T * C], dt_in, isOutput=False)
    out_ext = nc.declare_dram_parameter("out", [P, 2 * NT], f32, isOutput=True)

    add = mybir.AluOpType.add
    sub = mybir.AluOpType.subtract
    mult = mybir.AluOpType.mult

    with tile.TileContext(nc) as tc:
        with tc.tile_pool(name="consts", bufs=1) as consts, \
             tc.tile_pool(name="io", bufs=3) as io, \
             tc.tile_pool(name="work", bufs=3) as work, \
             tc.tile_pool(name="accp", bufs=1) as accp:
            vb = consts.tile([P, T * C], dt_in)
            nc.sync.dma_start(out=vb[:], in_=vb_ext[:])
            accs = accp.tile([P, 2 * NT], f32)
            # two persistent A19 buffers with the per-row 19th column
            # preset to 1.0 (ACT only ever rewrites columns 0:18)
            a19s = [consts.tile([P, T * C], dt_in, tag=f"a19_{j}", name=f"a19_{j}")
                    for j in range(2)]
            for j in range(2):
                nc.vector.memset(a19s[j][:], 1.0)

            for i in range(NT):
                mo_t = io.tile([P, T * C], dt_in, tag="mo")
                nc.gpsimd.dma_start(out=mo_t[:], in_=mo_ext[i])
                a_t = io.tile([P, T * NCOEF], dt_in, tag="a")
                nc.gpsimd.dma_start(out=a_t[:], in_=a_ext[i])
                y_t = io.tile([P, T], f32, tag="y")
                nc.sync.dma_start(out=y_t[:], in_=y_ext[i])
                b_t = io.tile([P, T], f32, tag="b")
                nc.sync.dma_start(out=b_t[:], in_=b_ext[i])

                bc = work.tile([P, T * C], dt_in, tag="bc")
                nc.vector.tensor_tensor(bc[:], mo_t[:], vb[:], add)

                a19 = a19s[i % 2]
                nc.scalar.copy(
                    out=a19[:].rearrange("p (t c) -> p t c", c=C)[:, :, 0:NCOEF],
                    in_=a_t[:].rearrange("p (t c) -> p t c", c=NCOEF),
                )
                nc.vector.tensor_tensor(bc[:], a19[:], bc[:], mult)

                tmp = work.tile([P, T], f32, tag="tmp")
                nc.vector.tensor_reduce(
                    tmp[:], bc[:].rearrange("p (t c) -> p t c", c=C),
                    axis=mybir.AxisListType.X, op=add,
                )

                # tensor_tensor_reduce (custom DVE op) crashes this runtime;
                # use sub/square/reduce instead
                e1 = work.tile([P, T], f32, tag="e1")
                nc.vector.tensor_tensor(e1[:], tmp[:], y_t[:], sub)
                nc.vector.tensor_tensor(e1[:], e1[:], e1[:], mult)
                nc.vector.tensor_reduce(
                    accs[:, 2 * i:2 * i + 1], e1[:],
                    axis=mybir.AxisListType.X, op=add,
                )
                e2 = work.tile([P, T], f32, tag="e2")
                nc.vector.tensor_tensor(e2[:], tmp[:], b_t[:], sub)
                nc.vector.tensor_tensor(e2[:], e2[:], e2[:], mult)
                nc.vector.tensor_reduce(
                    accs[:, 2 * i + 1:2 * i + 2], e2[:],
                    axis=mybir.AxisListType.X, op=add,
                )

            nc.sync.dma_start(out=out_ext[:], in_=accs[:])
    nc.finalize()
    _cached[key] = nc
    return nc


def _shard(arr, ncols):
    """Split [N_TOTAL, ncols] f32 into 8 shards of [NT, P, T*ncols]."""
    shards = []
    for i in range(N_CORES):
        lo = i * R
        hi = lo + R
        if hi <= N_TOTAL:
            s = arr[lo:hi]
        else:
            s = np.zeros((R, ncols), dtype=arr.dtype)
            s[: N_TOTAL - lo] = arr[lo:]
        shards.append(np.ascontiguousarray(s).reshape(NT, P, T * ncols))
    return shards


def _prepare_in_maps(model_output, y, A, B_tl, beta_TL):
    model_output = np.asarray(model_output, dtype=np.float32)
    y = np.asarray(y, dtype=np.float32)
    A = np.asarray(A, dtype=np.float32)
    B_tl = np.asarray(B_tl, dtype=np.float32)
    beta_TL = np.asarray(beta_TL, dtype=np.float32)

    vrow = np.concatenate([beta_TL, np.zeros(1, np.float32)])  # [19]
    dt_np = ml_dtypes.bfloat16 if USE_BF16 else np.float32
    vb = np.tile(vrow.astype(dt_np), (P, T))  # [128, T*19]

    mo_s = _shard(model_output, C)
    a_s = _shard(A, NCOEF)
    y_s = _shard(y, 1)
    b_s = _shard(B_tl, 1)
    return [
        {"mo": mo_s[i], "a": a_s[i], "y": y_s[i], "b": b_s[i], "vb": vb}
        for i in range(N_CORES)
    ]


def kernel(model_output, y, A, B_tl, beta_TL):
    nc = _build_nc()
    in_maps = _prepare_in_maps(model_output, y, A, B_tl, beta_TL)
    res = run_bass_kernel_spmd(nc, in_maps, list(range(N_CORES)))
    total = 0.0
    for r in res.results:
        total += float(r["out"].astype(np.float64).sum())
    return np.asarray(total / N_TOTAL, dtype=np.float32)


# revision 8
# speedup vs baseline: 262.5444x; 262.5444x over previous
"""Trainium2 Bass kernel for the Tolles-Lawson custom loss.

reference:
    c = model_output[:, :18]; d = model_output[:, 18:19]
    tmp = sum(A * (beta_TL + c), axis=1, keepdims=True) + d
    L = mean((tmp - y)^2) + mean((tmp - B_tl)^2)

Sharding: pure data parallel over rows on 8 cores. Each core gets
R = 501,760 rows (core 7 zero-padded; zero rows contribute 0 to both
sums). Rows are block-assigned to partitions so every DMA reads one
contiguous ~19KB per-partition run. Per-core partial sums [128, 2*NT]
are summed on the host and divided by N (the all-reduce of the two MSE
sums from the sharding hint, done host-side since the output is tiny).

Numerics: beta_TL must stay f32 — rounding it to bf16 is a *systematic*
per-coefficient bias across all 4M rows (~5e-4 relative on the loss),
while rounding A / model_output is row-random and washes out in the
mean (~5e-6). Default mode computes everything in f32.

default mode "f32_dma_accum": the beta+c add happens inside the SWDGE
DMA (accum_op=add) onto a tile the ACT engine pre-filled with
[beta,0]*T, so the DVE only does mult+reduce+small ops and stays under
the DMA roofline:
    bc   = vb_prefill (ACT) ; bc += mo (DMA accum)   # bc[...,18] = d
    prod = A * bc[:, :, :18]      (DVE)
    tmp  = reduce_sum(prod) + bc[:, :, 18]
    acc[2i], acc[2i+1] = sum((tmp-y)^2), sum((tmp-B)^2)
"""

import numpy as np
import ml_dtypes

import concourse.bacc as bacc
import concourse.mybir as mybir
from concourse import tile
from concourse.bass_utils import run_bass_kernel_spmd

N_TOTAL = 4_000_000
NCOEF = 18
C = NCOEF + 1  # 19: coeffs + bias column
P = 128
T = 245          # rows per partition per tile
NT = 16          # tiles per core
RP = T * NT      # 3920 rows per partition
R = P * RP       # 501,760 rows per core
N_CORES = 8

f32 = mybir.dt.float32
bf16 = mybir.dt.bfloat16

MODE = "f32_dma_accum"  # bf16 | f32_dma_accum | f32_dve | f32_gpsimd

_cached = {}


def _build_nc(T=T, NT=NT, mode=None, rep=1):
    mode = mode or MODE
    key = ("nc", T, NT, mode, rep)
    if key in _cached:
        return _cached[key]
    use_bf16 = mode == "bf16"
    dt_in = bf16 if use_bf16 else f32
    nc = bacc.Bacc(None)
    mo_ext = nc.declare_dram_parameter("mo", [NT, P, T * C], f32, isOutput=False)
    a_ext = nc.declare_dram_parameter("a", [NT, P, T * NCOEF], f32, isOutput=False)
    y_ext = nc.declare_dram_parameter("y", [NT, P, T], f32, isOutput=False)
    b_ext = nc.declare_dram_parameter("b", [NT, P, T], f32, isOutput=False)
    vb_ext = nc.declare_dram_parameter("vb", [P, T * C], dt_in, isOutput=False)
    out_ext = nc.declare_dram_parameter("out", [P, 2 * NT], f32, isOutput=True)

    add = mybir.AluOpType.add
    sub = mybir.AluOpType.subtract
    mult = mybir.AluOpType.mult
    AX = mybir.AxisListType.X

    with tile.TileContext(nc) as tc:
        with tc.tile_pool(name="consts", bufs=1) as consts, \
             tc.tile_pool(name="io", bufs=3) as io, \
             tc.tile_pool(name="work", bufs=3) as work, \
             tc.tile_pool(name="accp", bufs=1) as accp:
            vb = consts.tile([P, T * C], dt_in)
            nc.sync.dma_start(out=vb[:], in_=vb_ext[:])
            accs = accp.tile([P, 2 * NT], f32)

            if use_bf16:
                # two persistent [A | 1.0] buffers; col 18 preset to 1.0,
                # ACT rewrites cols 0:18 each tile
                a19s = [consts.tile([P, T * C], dt_in, tag=f"a19_{j}",
                                    name=f"a19_{j}") for j in range(2)]
                for j in range(2):
                    nc.vector.memset(a19s[j][:], 1.0)

            for r in range(rep):
                for i in range(NT):
                    a_t = io.tile([P, T * NCOEF], dt_in, tag="a")
                    if use_bf16:
                        nc.gpsimd.dma_start(out=a_t[:], in_=a_ext[i])
                    else:
                        nc.sync.dma_start(out=a_t[:], in_=a_ext[i])
                    y_t = io.tile([P, T], f32, tag="y")
                    nc.sync.dma_start(out=y_t[:], in_=y_ext[i])
                    b_t = io.tile([P, T], f32, tag="b")
                    nc.sync.dma_start(out=b_t[:], in_=b_ext[i])

                    bc = work.tile([P, T * C], dt_in, tag="bc")
                    if use_bf16:
                        mo_t = io.tile([P, T * C], dt_in, tag="mo")
                        nc.gpsimd.dma_start(out=mo_t[:], in_=mo_ext[i])
                        nc.vector.tensor_tensor(bc[:], mo_t[:], vb[:], add)
                    elif mode == "f32_dma_accum":
                        nc.scalar.copy(out=bc[:], in_=vb[:])
                        # CCE (inline DMA accumulate ALU) handles at most
                        # 2048 elements per descriptor; larger runs are
                        # silently wrong, and max_dma_last_dim is not
                        # honored for Tile's symbolic APs — slice manually
                        w = T * C
                        nslice = -(-w // 2048)
                        step = -(-w // nslice)
                        step += (-step) % 8  # keep 32B-aligned slice starts
                        for s0 in range(0, w, step):
                            s1 = min(s0 + step, w)
                            nc.gpsimd.dma_start(out=bc[:, s0:s1],
                                                in_=mo_ext[i][:, s0:s1],
                                                accum_op=add)
                    else:
                        nc.sync.dma_start(out=bc[:], in_=mo_ext[i])
                        eng = nc.vector if mode == "f32_dve" else nc.gpsimd
                        eng.tensor_tensor(bc[:], bc[:], vb[:], add)

                    bc3 = bc[:].rearrange("p (t c) -> p t c", c=C)
                    tmp = work.tile([P, T], f32, tag="tmp")
                    if use_bf16:
                        a19 = a19s[i % 2]
                        nc.scalar.copy(
                            out=a19[:].rearrange("p (t c) -> p t c", c=C)[:, :, 0:NCOEF],
                            in_=a_t[:].rearrange("p (t c) -> p t c", c=NCOEF),
                        )
                        nc.vector.tensor_tensor(bc[:], a19[:], bc[:], mult)
                        nc.vector.tensor_reduce(tmp[:], bc3, axis=AX, op=add)
                    else:
                        prod = work.tile([P, T * NCOEF], dt_in, tag="prod")
                        nc.vector.tensor_tensor(
                            prod[:], a_t[:], bc3[:, :, 0:NCOEF], mult)
                        nc.vector.tensor_reduce(
                            tmp[:], prod[:].rearrange("p (t c) -> p t c", c=NCOEF),
                            axis=AX, op=add)
                        nc.vector.tensor_tensor(tmp[:], tmp[:], bc3[:, :, NCOEF], add)

                    e1 = work.tile([P, T], f32, tag="e1")
                    nc.vector.tensor_tensor(e1[:], tmp[:], y_t[:], sub)
                    nc.vector.tensor_tensor(e1[:], e1[:], e1[:], mult)
                    nc.vector.tensor_reduce(
                        accs[:, 2 * i:2 * i + 1], e1[:], axis=AX, op=add)
                    e2 = work.tile([P, T], f32, tag="e2")
                    nc.vector.tensor_tensor(e2[:], tmp[:], b_t[:], sub)
                    nc.vector.tensor_tensor(e2[:], e2[:], e2[:], mult)
                    nc.vector.tensor_reduce(
                        accs[:, 2 * i + 1:2 * i + 2], e2[:], axis=AX, op=add)

            nc.sync.dma_start(out=out_ext[:], in_=accs[:])
    nc.finalize()
    _cached[key] = nc
    return nc


def _shard(arr, ncols):
    """Split [N_TOTAL, ncols] f32 into 8 shards of [NT, P, T*ncols]."""
    shards = []
    for i in range(N_CORES):
        lo = i * R
        hi = lo + R
        if hi <= N_TOTAL:
            s = arr[lo:hi]
        else:
            s = np.zeros((R, ncols), dtype=arr.dtype)
            s[: N_TOTAL - lo] = arr[lo:]
        shards.append(np.ascontiguousarray(s).reshape(NT, P, T * ncols))
    return shards


def _prepare_in_maps(model_output, y, A, B_tl, beta_TL, mode=None):
    mode = mode or MODE
    model_output = np.asarray(model_output, dtype=np.float32)
    y = np.asarray(y, dtype=np.float32)
    A = np.asarray(A, dtype=np.float32)
    B_tl = np.asarray(B_tl, dtype=np.float32)
    beta_TL = np.asarray(beta_TL, dtype=np.float32)

    vrow = np.concatenate([beta_TL, np.zeros(1, np.float32)])  # [19]
    dt_np = ml_dtypes.bfloat16 if mode == "bf16" else np.float32
    vb = np.tile(vrow.astype(dt_np), (P, T))  # [128, T*19]

    mo_s = _shard(model_output, C)
    a_s = _shard(A, NCOEF)
    y_s = _shard(y, 1)
    b_s = _shard(B_tl, 1)
    return [
        {"mo": mo_s[i], "a": a_s[i], "y": y_s[i], "b": b_s[i], "vb": vb}
        for i in range(N_CORES)
    ]


def kernel(model_output, y, A, B_tl, beta_TL):
    nc = _build_nc()
    in_maps = _prepare_in_maps(model_output, y, A, B_tl, beta_TL)
    res = run_bass_kernel_spmd(nc, in_maps, list(range(N_CORES)))
    total = 0.0
    for r in res.results:
        total += float(r["out"].astype(np.float64).sum())
    return np.asarray(total / N_TOTAL, dtype=np.float32)
